# revision 22
# baseline (speedup 1.0000x reference)
"""Trainium2 Bass kernel for EnhancedGraphTransformerLayer.

Layer: LN1 -> QKV proj -> per-node 8x8 head attention -> O proj -> residual
       -> LN2 -> FFN(512->2048->512, relu) -> residual.

Strategy (per NeuronCore, data-parallel over nodes, 8 cores):
- All big matmuls in bf16 on the PE (fp32 accumulate in PSUM), activations
  flow feature-transposed ([feature, node]) with weights stationary.
- Projections run at chunk granularity (512 nodes) with N=512 moving
  operands to keep the PE warm and amortize LDWEIGHTS.
- Per-node 8-head attention uses 16-node sub-group packing WITHOUT any
  data-movement gather: the (128,128) score matrix for a sub-group is
  assembled from 4 quadrant matmuls (K=64) whose operands are strided APs
  directly into the projection outputs. A host-side column-swapped copy of
  Wk ("rwkw") provides k-heads of either parity at either partition half so
  stationary/moving partition bases always match.
- A block-diagonal mask (multiplied on GpSimd) zeroes cross-node terms
  after exp; an appended ones-column of V yields softmax denominators
  inside the AV matmul.
- Attention-output transposes are split by head parity so the O projection
  contracts over K=128 (full PE array) in 4 matmuls.
- LayerNorm stats via bn_stats/bn_aggr in natural layout; gamma/beta are
  folded into weights/biases on the host.
"""

import numpy as np
import ml_dtypes
from contextlib import ExitStack

F8 = ml_dtypes.float8_e4m3fn

E = 512
H = 8
D = 64
F = 2048
EPS = 1e-5
N_NODES = 65536
N_CORES = 8
BF = ml_dtypes.bfloat16


def build_nc(npc, has_qkv_bias=False, has_bo=False, has_c2f=False,
             has_b2=False, fix_waits=True):
    import concourse.bass as bass
    import concourse.mybir as mybir

    f32 = mybir.dt.float32
    bf16 = mybir.dt.bfloat16
    f8 = mybir.dt.float8e4

    nc = bass.Bass()
    ins = dict(
        x=nc.dram_tensor("x", (npc, E), f32, kind="ExternalInput").ap(),
        rwq=nc.dram_tensor("rwq", (E, E), bf16, kind="ExternalInput").ap(),
        rwk=nc.dram_tensor("rwk", (E, E), bf16, kind="ExternalInput").ap(),
        rwkw=nc.dram_tensor("rwkw", (E, E), bf16, kind="ExternalInput").ap(),
        rwv=nc.dram_tensor("rwv", (E, E), bf16, kind="ExternalInput").ap(),
        rwo2=nc.dram_tensor("rwo2", (E, E), bf16, kind="ExternalInput").ap(),
        rw1=nc.dram_tensor("rw1", (E, F), f8, kind="ExternalInput").ap(),
        w2t=nc.dram_tensor("w2t", (F, E), f8, kind="ExternalInput").ap(),
        mask=nc.dram_tensor("mask", (128, 128), bf16, kind="ExternalInput").ap(),
        c2q=nc.dram_tensor("c2q", (E,), f32, kind="ExternalInput").ap(),
        c2k=nc.dram_tensor("c2k", (E,), f32, kind="ExternalInput").ap(),
        c2kw=nc.dram_tensor("c2kw", (E,), f32, kind="ExternalInput").ap(),
        c2v=nc.dram_tensor("c2v", (E,), f32, kind="ExternalInput").ap(),
        bo=nc.dram_tensor("bo", (E,), bf16, kind="ExternalInput").ap(),
        c2f=nc.dram_tensor("c2f", (F,), f32, kind="ExternalInput").ap(),
        b2=nc.dram_tensor("b2", (E,), f32, kind="ExternalInput").ap(),
    )
    out_ap = nc.dram_tensor("out", (npc, E), f32, kind="ExternalOutput").ap()
    build_body(nc, ins, out_ap, npc, has_qkv_bias=has_qkv_bias,
               has_bo=has_bo, has_c2f=has_c2f, has_b2=has_b2,
               fix_waits=fix_waits)
    return nc


def build_body(nc, ins, out_d, npc, has_qkv_bias=False, has_bo=False,
               has_c2f=False, has_b2=False, fix_waits=True):
    import concourse.bass as bass
    import concourse.mybir as mybir
    from concourse.tile import TileContext
    from concourse.masks import make_identity

    f32 = mybir.dt.float32
    bf16 = mybir.dt.bfloat16
    f8 = mybir.dt.float8e4
    AL = mybir.AluOpType
    AF = mybir.ActivationFunctionType
    DR = mybir.MatmulPerfMode.DoubleRow

    n_groups = npc // 128
    gpc = 4 if n_groups % 4 == 0 else 1  # groups per chunk
    n_chunks = n_groups // gpc
    npcch = 128 * gpc  # nodes per chunk
    nsb = npcch // 16  # 16-node sub-groups per chunk

    x_d = ins["x"]
    rwq_d, rwk_d, rwkw_d, rwv_d = ins["rwq"], ins["rwk"], ins["rwkw"], ins["rwv"]
    rwo2_d, rw1_d, w2t_d, mask_d = ins["rwo2"], ins["rw1"], ins["w2t"], ins["mask"]
    c2q_d, c2k_d, c2kw_d, c2v_d = ins["c2q"], ins["c2k"], ins["c2kw"], ins["c2v"]
    bo_d, c2f_d, b2_d = ins["bo"], ins["c2f"], ins["b2"]

    with TileContext(nc) as tc, ExitStack() as ctx:
        wpool = ctx.enter_context(tc.tile_pool(name="w", bufs=1))
        pool = ctx.enter_context(tc.tile_pool(name="act", bufs=1))
        psum = ctx.enter_context(tc.tile_pool(name="ps", bufs=1, space="PSUM"))

        # ---- constants / weights ----
        rwq_sb = wpool.tile([128, 4, E], bf16, tag="rwq")
        rwk_sb = wpool.tile([128, 4, E], bf16, tag="rwk")
        rwkw_sb = wpool.tile([128, 4, E], bf16, tag="rwkw")
        rwv_sb = wpool.tile([128, 4, E], bf16, tag="rwv")
        nc.sync.dma_start(out=rwq_sb, in_=rwq_d.rearrange("(t p) e -> p t e", p=128))
        nc.sync.dma_start(out=rwk_sb, in_=rwk_d.rearrange("(t p) e -> p t e", p=128))
        nc.sync.dma_start(out=rwkw_sb, in_=rwkw_d.rearrange("(t p) e -> p t e", p=128))
        nc.sync.dma_start(out=rwv_sb, in_=rwv_d.rearrange("(t p) e -> p t e", p=128))
        # rwo2[(h2*64+d), h1, fo] = Wo[fo, (2*h1+h2)*64+d]
        rwo2_sb = wpool.tile([128, 4, E], bf16, tag="rwo2")
        nc.scalar.dma_start(out=rwo2_sb, in_=rwo2_d.rearrange("(p t) e -> p t e", t=4))
        rw1_sb = wpool.tile([128, 4, F], f8, tag="rw1")
        nc.scalar.dma_start(out=rw1_sb, in_=rw1_d.rearrange("(t p) f -> p t f", p=128))
        w2t_sb = wpool.tile([128, 16, E], f8, tag="w2t")
        nc.scalar.dma_start(out=w2t_sb, in_=w2t_d.rearrange("(t p) e -> p t e", p=128))
        mask_sb = wpool.tile([128, 128], bf16, tag="mask")
        nc.sync.dma_start(out=mask_sb, in_=mask_d)
        ident128 = wpool.tile([128, 128], bf16, tag="id128")
        make_identity(nc, ident128)
        eps_sb = wpool.tile([128, 1], f32, tag="eps")
        nc.vector.memset(eps_sb, EPS)
        vaugA = wpool.tile([128, 4, 66], bf16, tag="vaugA")
        vaugB = wpool.tile([128, 4, 66], bf16, tag="vaugB")
        nc.vector.memset(vaugA[:, :, 64:65], 1.0)
        nc.vector.memset(vaugB[:, :, 64:65], 1.0)
        if has_qkv_bias:
            c2q_sb = wpool.tile([128, 4], f32, tag="c2q")
            c2k_sb = wpool.tile([128, 4], f32, tag="c2k")
            c2kw_sb = wpool.tile([128, 4], f32, tag="c2kw")
            c2v_sb = wpool.tile([128, 4], f32, tag="c2v")
            nc.sync.dma_start(out=c2q_sb, in_=c2q_d.rearrange("(t p) -> p t", p=128))
            nc.sync.dma_start(out=c2k_sb, in_=c2k_d.rearrange("(t p) -> p t", p=128))
            nc.sync.dma_start(out=c2kw_sb, in_=c2kw_d.rearrange("(t p) -> p t", p=128))
            nc.sync.dma_start(out=c2v_sb, in_=c2v_d.rearrange("(t p) -> p t", p=128))
        if has_bo:
            ones1_sb = wpool.tile([1, 128], bf16, tag="ones1")
            nc.vector.memset(ones1_sb, 1.0)
            bo_sb = wpool.tile([1, E], bf16, tag="bo")
            nc.sync.dma_start(out=bo_sb, in_=bo_d.rearrange("(o e) -> o e", o=1))
        if has_c2f:
            c2f_sb = wpool.tile([128, 16], f32, tag="c2f")
            nc.sync.dma_start(out=c2f_sb, in_=c2f_d.rearrange("(t p) -> p t", p=128))
        if has_b2:
            b2_sb = wpool.tile([128, 4], f32, tag="b2")
            nc.sync.dma_start(out=b2_sb, in_=b2_d.rearrange("(t p) -> p t", p=128))

        def apx(tile_ap, off, dims):
            """Custom AP into tile at flat-element offset `off`."""
            return bass.AP(tensor=tile_ap.tensor, offset=tile_ap.offset + off,
                           ap=[list(d) for d in dims])

        def bcast_u(small, n_u, n_d):
            """(128, n_u) AP broadcast to (128, n_u, n_d) via stride-0."""
            return bass.AP(tensor=small.tensor, offset=small.offset,
                           ap=[small.ap[0], [1, n_u], [0, n_d]])

        def layernorm_to_bf16(x_sb, tagp):
            stat = pool.tile([128, 6], f32, tag=tagp + "stat", bufs=2, name=tagp + "stat")
            nc.vector.bn_stats(out=stat, in_=x_sb)
            mv = pool.tile([128, 2], f32, tag=tagp + "mv", bufs=2, name=tagp + "mv")
            nc.vector.bn_aggr(out=mv, in_=stat)
            rs = pool.tile([128, 1], f32, tag=tagp + "rs", bufs=2, name=tagp + "rs")
            nc.scalar.activation(out=rs, in_=mv[:, 1:2], func=AF.Sqrt,
                                 bias=eps_sb, scale=1.0)
            nc.vector.reciprocal(out=rs, in_=rs)
            zb = pool.tile([128, E], bf16, tag=tagp + "zb", bufs=2, name=tagp + "zb")
            nc.vector.tensor_scalar(out=zb, in0=x_sb, scalar1=mv[:, 0:1],
                                    scalar2=rs, op0=AL.subtract, op1=AL.mult)
            return zb

        QP = 4 * E      # per-partition element pitch of [128, 4, E] tiles

        def emit_ln1_group(c, gi):
            """x load + LN1 DVE chain for one group (no PE work)."""
            g = c * gpc + gi
            x_sb = pool.tile([128, E], f32, tag="x", bufs=2 * gpc,
                             name="x_sb")
            nc.sync.dma_start(out=x_sb, in_=x_d[g * 128:(g + 1) * 128, :])
            return x_sb, layernorm_to_bf16(x_sb, "ln1")

        def emit_ln1_loads(c):
            x_tiles, zb_tiles = [], []
            for gi in range(gpc):
                x_sb, zb = emit_ln1_group(c, gi)
                x_tiles.append(x_sb)
                zb_tiles.append(zb)
            return x_tiles, zb_tiles

        def emit_zbT(zb_tiles):
            """PE transposes of LN1 outputs into chunk-level zbT."""
            zbT_ch = pool.tile([128, 4, npcch], bf16, tag="zbT", bufs=2,
                               name="zbT_ch")
            for gi in range(gpc):
                zbT_ps = psum.tile([128, 4, 128], bf16, tag="tp", bufs=2,
                                   name="zbT_ps")
                for tau in range(4):
                    nc.tensor.transpose(zbT_ps[:, tau, :],
                                        zb_tiles[gi][:, 128 * tau:128 * (tau + 1)],
                                        ident128[:, :])
                dst = zbT_ch[:, :, gi * 128:(gi + 1) * 128]
                if gi % 2 == 0:
                    nc.vector.tensor_copy(out=dst, in_=zbT_ps)
                else:
                    nc.scalar.activation(out=dst, in_=zbT_ps, func=AF.Copy)
            return zbT_ch

        def emit_chunk(c, x_tiles, zbT_ch, fillers=(), next_c=None):
            # ---- Q/K/Kw/V projections (feature-transposed, chunk-wide) ----
            # SBUF layout [128, nsb, 64]: col (sb, tau*16+j) so each
            # sub-group's matmul operand is one contiguous 64-col run
            # (matmul moving APs allow only one free dimension).
            qTb = pool.tile([128, nsb, 64], bf16, tag="qTb", bufs=2, name="qTb")
            kTb = pool.tile([128, nsb, 64], bf16, tag="kTb", bufs=2, name="kTb")
            kTw = pool.tile([128, nsb, 64], bf16, tag="kTw", bufs=2, name="kTw")
            vTb = pool.tile([128, nsb, 64], bf16, tag="vTb", bufs=2, name="vTb")
            projs = [(rwq_sb, qTb, "c2q", 0), (rwk_sb, kTb, "c2k", 1),
                     (rwkw_sb, kTw, "c2kw", 0), (rwv_sb, vTb, "c2v", 1)]
            if has_qkv_bias:
                cmap = dict(c2q=c2q_sb, c2k=c2k_sb, c2kw=c2kw_sb, c2v=c2v_sb)
            for tau in range(4):
                for rw_sb, dst, cname, on_scalar in projs:
                    pp = psum.tile([128, npcch], f32, tag="big", bufs=3, name="pp")
                    for et in range(4):
                        nc.tensor.matmul(pp,
                                         rw_sb[:, et, 128 * tau:128 * (tau + 1)],
                                         zbT_ch[:, et, :],
                                         start=(et == 0), stop=(et == 3))
                    # dst cols (sb: stride 64) x (j: 16) at offset tau*16
                    dst_ap = apx(dst, tau * 16,
                                 [[nsb * 64, 128], [64, nsb], [1, 16]])
                    if has_qkv_bias:
                        cs = cmap[cname]
                        if on_scalar:
                            nc.scalar.activation(out=dst_ap, in_=pp,
                                                 func=AF.Identity,
                                                 bias=cs[:, tau:tau + 1])
                        else:
                            nc.vector.tensor_scalar_add(out=dst_ap,
                                                        in0=pp,
                                                        scalar1=cs[:, tau:tau + 1])
                    elif on_scalar:
                        nc.scalar.activation(out=dst_ap, in_=pp,
                                             func=AF.Copy)
                    else:
                        nc.vector.tensor_copy(out=dst_ap, in_=pp)

            # ---- attention per group (two-phase emission keeps PE fed) ----
            x2_tiles = []
            nxt = []
            z2bT_ch = pool.tile([128, 4, npcch], f8, tag="z2bT",
                                bufs=2, name="z2bT_ch")
            mask_b = bass.AP(tensor=mask_sb.tensor, offset=mask_sb.offset,
                             ap=[list(mask_sb.ap[0]), [0, 4], [1, 128]])
            pend_z2b = None  # (z2b, gi) transposed one group later

            def emit_z2bT(z2b, gi):
                z2bT_ps = psum.tile([128, 4, 128], bf16, tag="tp", bufs=2,
                                    name="z2bT_ps")
                for tau in range(4):
                    nc.tensor.transpose(z2bT_ps[:, tau, :],
                                        z2b[:, 128 * tau:128 * (tau + 1)],
                                        ident128[:, :])
                nc.scalar.activation(out=z2bT_ch[:, :, gi * 128:(gi + 1) * 128],
                                     in_=z2bT_ps, func=AF.Copy)

            for gi in range(gpc):
                pg2 = pool.tile([128, 4, 128], bf16, tag="pg2", bufs=2,
                                name="pg2")
                halves = []
                for half in range(2):
                    sb0 = gi * 8 + half * 4
                    # scores: 4 quadrant matmuls per sub-group. Split into
                    # two PSUM tiles by column half (h2) so concurrent MMs
                    # on different PE row-groups never drain to the same
                    # PSUM partitions of one bank (HW write collision).
                    s2L = psum.tile([128, 4, 64], f32, tag="att", bufs=3,
                                    name="s2L")
                    s2R = psum.tile([128, 4, 64], f32, tag="att", bufs=3,
                                    name="s2R")
                    for u in range(4):
                        sb = sb0 + u
                        for g2, h2 in ((0, 0), (1, 1), (1, 0), (0, 1)):
                            kX = kTb if g2 == h2 else kTw
                            off = h2 * 64 * QP + sb * 64
                            stat = apx(kX, off, [[QP, 64], [1, 64]])
                            mov = apx(qTb, off, [[QP, 64], [1, 64]])
                            s2 = s2L if h2 == 0 else s2R
                            nc.tensor.matmul(
                                s2[g2 * 64:(g2 + 1) * 64, u, :],
                                stat, mov, start=True, stop=True)
                    e_sb = pool.tile([128, 4, 128], bf16, tag="esb", bufs=2,
                                     name="e_sb")
                    eL = apx(e_sb, 0, [[512, 128], [128, 4], [1, 64]])
                    eR = apx(e_sb, 64, [[512, 128], [128, 4], [1, 64]])
                    nc.scalar.activation(out=eL, in_=s2L, func=AF.Exp)
                    nc.scalar.activation(out=eR, in_=s2R, func=AF.Exp)
                    a_sb = pool.tile([128, 4, 128], bf16, tag="asb", bufs=2,
                                     name="a_sb")
                    nc.gpsimd.tensor_tensor(out=a_sb, in0=e_sb, in1=mask_b,
                                            op=AL.mult)
                    # v sub-group packing via per-half transposes
                    vt_ps = psum.tile([128, 4, 64], bf16, tag="tp", bufs=2,
                                      name="vt_ps")
                    for u in range(4):
                        sb = sb0 + u
                        for g2 in range(2):
                            src = apx(vTb, g2 * 64 * QP + sb * 64,
                                      [[QP, 64], [1, 64]])
                            nc.tensor.transpose(
                                vt_ps[g2 * 64:(g2 + 1) * 64, u, :], src,
                                ident128[g2 * 64:(g2 + 1) * 64,
                                         g2 * 64:(g2 + 1) * 64])
                    vaug = vaugA if half == 0 else vaugB
                    if half == 0:
                        nc.scalar.activation(out=vaug[:, :, 0:64], in_=vt_ps,
                                             func=AF.Copy)
                    else:
                        nc.vector.tensor_copy(out=vaug[:, :, 0:64], in_=vt_ps)
                    halves.append((a_sb, vaug))

                for half in range(2):
                    a_sb, vaug = halves[half]
                    # AV (+denominator)
                    outS = psum.tile([128, 4, 66], f32, tag="att", bufs=3,
                                     name="outS")
                    for u in range(4):
                        nc.tensor.matmul(outS[:, u, 0:65], a_sb[:, u, :],
                                         vaug[:, u, 0:65], start=True,
                                         stop=True)
                    # normalize
                    recip = pool.tile([128, 4], f32, tag="recip", bufs=2,
                                      name="recip")
                    nc.vector.reciprocal(out=recip, in_=outS[:, :, 64])
                    ogb = pool.tile([128, 4, 64], bf16, tag="ogb", bufs=2,
                                    name="ogb")
                    nc.vector.tensor_tensor(out=ogb, in0=outS[:, :, 0:64],
                                            in1=bcast_u(recip, 4, 64),
                                            op=AL.mult)
                    # out transposes split by head parity
                    # -> pg2[(h2,d), h1, (sb,j)]
                    p_ps = psum.tile([128, 4, 64], bf16, tag="tp", bufs=2,
                                     name="p_ps")
                    for u in range(4):
                        for h2 in range(2):
                            nc.tensor.transpose(
                                p_ps[h2 * 64:(h2 + 1) * 64, u, :],
                                ogb[h2 * 64:(h2 + 1) * 64, u, :],
                                ident128[h2 * 64:(h2 + 1) * 64,
                                         h2 * 64:(h2 + 1) * 64])
                    # pg2 col = h1*128 + sb*16 + j (per-h1 contiguous for
                    # the single-free-dim O-proj stationary)
                    pg2_ap = apx(pg2, half * 64,
                                 [[512, 128], [16, 4], [128, 4], [1, 16]])
                    nc.scalar.activation(out=pg2_ap, in_=p_ps, func=AF.Copy)

                # ---- O projection: K=128 over (h2, d), accumulate over h1 ----
                oproj_ps = psum.tile([128, E], f32, tag="big", bufs=3,
                                     name="oproj_ps")
                for h1 in range(4):
                    nc.tensor.matmul(oproj_ps, pg2[:, h1, :], rwo2_sb[:, h1, :],
                                     start=(h1 == 0),
                                     stop=(h1 == 3 and not has_bo))
                if has_bo:
                    nc.tensor.matmul(oproj_ps, ones1_sb, bo_sb,
                                     start=False, stop=True)

                # ---- residual 1 + LN2 (z2bT transposes deferred a group) ----
                x2_sb = pool.tile([128, E], f32, tag="x2", bufs=2 * gpc + 2,
                                  name="x2_sb")
                nc.vector.tensor_add(out=x2_sb, in0=x_tiles[gi], in1=oproj_ps)
                x2_tiles.append(x2_sb)
                z2b = layernorm_to_bf16(x2_sb, "ln2")
                if pend_z2b is not None:
                    emit_z2bT(*pend_z2b)
                pend_z2b = (z2b, gi)
                # next chunk's x load + LN1 chain, spread across groups
                if next_c is not None:
                    nxt.append(emit_ln1_group(next_c, gi))
                # dense-MM filler (prev chunk's FFN slice) keeps the PE's
                # HAM clock warm through the transpose-heavy attention
                if gi < len(fillers):
                    fillers[gi]()

            if len(fillers) > 4:
                fillers[4]()
                emit_z2bT(*pend_z2b)
                fillers[5]()
            else:
                emit_z2bT(*pend_z2b)
            return x2_tiles, z2bT_ch, nxt

        def make_ffn_pieces(c, z2bT_ch, x2_tiles):
            """FFN for chunk c as 6 closures: 4x (u1 ft block), u2, tail."""
            cell = {}

            def u1_block(ft0):
                def go():
                    if ft0 == 0:
                        cell["rT"] = pool.tile([128, 16, npcch], f8,
                                               tag="rt", bufs=1, name="rT_sb")
                    rT_sb = cell["rT"]
                    for ft in range(ft0, ft0 + 4):
                        u1_ps = psum.tile([128, npcch], f32, tag="big",
                                          bufs=3, name="u1_ps")
                        for t2 in range(2):
                            stat = apx(rw1_sb, t2 * 2 * F + ft * 128,
                                       [[4 * F, 128], [F, 2], [1, 128]])
                            mov = apx(z2bT_ch, t2 * 2 * npcch,
                                      [[4 * npcch, 128], [npcch, 2],
                                       [1, npcch]])
                            nc.tensor.matmul(u1_ps, stat, mov, perf_mode=DR,
                                             start=(t2 == 0), stop=(t2 == 1))
                        if has_c2f:
                            nc.vector.tensor_scalar(
                                out=rT_sb[:, ft, :], in0=u1_ps,
                                scalar1=c2f_sb[:, ft:ft + 1],
                                scalar2=0.0, op0=AL.add, op1=AL.max)
                        elif ft % 2 == 0:
                            nc.scalar.activation(out=rT_sb[:, ft, :],
                                                 in_=u1_ps, func=AF.Relu)
                        else:
                            nc.vector.tensor_scalar_max(out=rT_sb[:, ft, :],
                                                        in0=u1_ps, scalar1=0.0)
                return go

            def u2_all():
                rT_sb = cell["rT"]
                u2b_sb = pool.tile([128, 4, npcch], bf16, tag="u2b", bufs=2,
                                   name="u2b_sb")
                cell["u2b"] = u2b_sb
                for et in range(4):
                    u2_ps = psum.tile([128, npcch], f32, tag="big", bufs=3,
                                      name="u2_ps")
                    for t2 in range(8):
                        stat = apx(w2t_sb, t2 * 2 * E + et * 128,
                                   [[16 * E, 128], [E, 2], [1, 128]])
                        mov = apx(rT_sb, t2 * 2 * npcch,
                                  [[16 * npcch, 128], [npcch, 2], [1, npcch]])
                        nc.tensor.matmul(u2_ps, stat, mov, perf_mode=DR,
                                         start=(t2 == 0), stop=(t2 == 7))
                    if has_b2:
                        nc.vector.tensor_scalar_add(
                            out=u2b_sb[:, et, :], in0=u2_ps,
                            scalar1=b2_sb[:, et:et + 1])
                    elif et % 2 == 0:
                        nc.scalar.activation(out=u2b_sb[:, et, :], in_=u2_ps,
                                             func=AF.Copy)
                    else:
                        nc.vector.tensor_copy(out=u2b_sb[:, et, :], in_=u2_ps)

            def tail():
                u2b_sb = cell["u2b"]
                for gi in range(gpc):
                    g = c * gpc + gi
                    u2n_ps = psum.tile([128, 4, 128], bf16, tag="tp", bufs=2,
                                       name="u2n_ps")
                    for et in range(4):
                        nc.tensor.transpose(
                            u2n_ps[:, et, :],
                            u2b_sb[:, et, 128 * gi:128 * (gi + 1)],
                            ident128[:, :])
                    u2nat = pool.tile([128, 4, 128], bf16, tag="u2nat",
                                      bufs=2, name="u2nat")
                    if gi % 2 == 0:
                        nc.vector.tensor_copy(out=u2nat, in_=u2n_ps)
                    else:
                        nc.scalar.activation(out=u2nat, in_=u2n_ps,
                                             func=AF.Copy)
                    out_sb = pool.tile([128, E], f32, tag="osb", bufs=3,
                                       name="out_sb")
                    nc.gpsimd.tensor_add(out=out_sb, in0=x2_tiles[gi],
                                         in1=u2nat)
                    nc.sync.dma_start(out=out_d[g * 128:(g + 1) * 128, :],
                                      in_=out_sb)

            return [u1_block(0), u1_block(4), u1_block(8), u1_block(12),
                    u2_all, tail]

        # ---- software-pipelined main loop (chunk-skewed, interleaved FFN) --
        x_next, zb_next = emit_ln1_loads(0)
        zbT_next = emit_zbT(zb_next)
        fillers = ()
        for c in range(n_chunks):
            x_cur, zbT_cur = x_next, zbT_next
            nc_arg = c + 1 if c + 1 < n_chunks else None
            x2_tiles, z2bT_ch, nxt = emit_chunk(c, x_cur, zbT_cur, fillers,
                                                next_c=nc_arg)
            if nc_arg is not None:
                x_next = [t[0] for t in nxt]
                zbT_next = emit_zbT([t[1] for t in nxt])
            fillers = make_ffn_pieces(c, z2bT_ch, x2_tiles)
        for piece in fillers:
            piece()

    if fix_waits:
        _fix_sync_waits(nc)


def _fix_sync_waits(nc):
    """walrus limits inline sync waits to 1 per instruction. Tile can
    emit more. Split the excess into standalone InstEventSemaphore
    wait-carriers inserted immediately before the overweight instruction
    on the same engine - semantically identical (the waits still execute
    right before the instruction, in order)."""
    import concourse.mybir as mybir
    n = 0
    for f in nc.m.functions:
        for blk in f.blocks:
            insts = blk.instructions
            out = []
            dirty = False
            for inst in insts:
                si = inst.sync_info
                waits = list(si.on_wait) if (si and si.on_wait) else []
                limit = 1
                if len(waits) > limit:
                    ups = list(si.on_update) if (si and si.on_update) else []
                    up_ids = {u.id for u in ups}
                    # keep own-queue credit waits inline (DMA flow control)
                    waits.sort(key=lambda w: 0 if w.id in up_ids else 1)
                    keep, move = waits[:limit], waits[limit:]
                    for w in move:
                        n += 1
                        car = mybir.InstEventSemaphore(
                            name="WSPLIT-%d" % n, ins=[], outs=[])
                        car.engine = inst.engine
                        car.sync_info = mybir.SyncInfo(on_wait=[w],
                                                       on_update=[])
                        out.append(car)
                    inst.sync_info = mybir.SyncInfo(on_wait=keep,
                                                   on_update=ups)
                    dirty = True
                out.append(inst)
            if dirty:
                blk.instructions = out
    return n


def _prep_weights(inputs):
    """Host-side weight folding. Returns dict of np arrays + flags."""
    f32 = np.float32
    g1 = np.asarray(inputs["g1"], f32)
    beta1 = np.asarray(inputs["beta1"], f32)
    g2 = np.asarray(inputs["g2"], f32)
    beta2 = np.asarray(inputs["beta2"], f32)
    Wq = np.asarray(inputs["Wq"], f32)
    Wk = np.asarray(inputs["Wk"], f32)
    Wv = np.asarray(inputs["Wv"], f32)
    Wo = np.asarray(inputs["Wo"], f32)
    W1 = np.asarray(inputs["W1"], f32)
    W2 = np.asarray(inputs["W2"], f32)
    scale = np.float32(1.0 / np.sqrt(D))

    rwq = (Wq.T * g1[:, None] * scale).astype(BF)
    rwk = (Wk.T * g1[:, None]).astype(BF)
    rwv = (Wv.T * g1[:, None]).astype(BF)
    rw1 = (W1.T * g2[:, None]).astype(F8)
    w2t = W2.T.astype(F8)

    # column-swapped K weights: within each 128-col block, swap 64-halves
    sw = np.arange(E)
    sw = (sw // 128) * 128 + (sw % 128 + 64) % 128
    rwkw = np.ascontiguousarray(rwk[:, sw])

    # rwo2[(h2*64+d), h1, fo] = Wo[fo, (2*h1+h2)*64+d]  (stored [E, E] with
    # row index (h2*64+d)*4... laid out as [(p t) e] with p=(h2*64+d), t=h1)
    WoT = Wo.T.astype(BF)  # [E(h*64+d), E(fo)]
    rwo2 = np.zeros((E, E), BF)
    for h2 in range(2):
        for h1 in range(4):
            h = 2 * h1 + h2
            # destination rows (h2*64+d) at t=h1: flat row = (h2*64+d)*4 + h1
            rwo2[(np.arange(64) + h2 * 64) * 4 + h1, :] = WoT[h * 64:(h + 1) * 64, :]

    c2q = ((Wq @ beta1 + np.asarray(inputs["bq"], f32)) * scale).astype(f32)
    c2k = (Wk @ beta1 + np.asarray(inputs["bk"], f32)).astype(f32)
    c2kw = np.ascontiguousarray(c2k[sw])
    c2v = (Wv @ beta1 + np.asarray(inputs["bv"], f32)).astype(f32)
    bo = np.asarray(inputs["bo"], f32)
    c2f = (W1 @ beta2 + np.asarray(inputs["b1"], f32)).astype(f32)
    b2 = np.asarray(inputs["b2"], f32)

    mask = np.zeros((128, 128), f32)
    for i in range(16):
        for gg in range(8):
            for hh in range(8):
                mask[gg * 16 + i, hh * 16 + i] = 1.0

    return dict(
        rwq=rwq, rwk=rwk, rwkw=rwkw, rwv=rwv, rwo2=rwo2, rw1=rw1, w2t=w2t,
        mask=mask.astype(BF),
        c2q=c2q, c2k=c2k, c2kw=c2kw, c2v=c2v, bo=bo.astype(BF), c2f=c2f, b2=b2,
        has_qkv_bias=bool(np.any(c2q) or np.any(c2k) or np.any(c2v)),
        has_bo=bool(np.any(bo)), has_c2f=bool(np.any(c2f)),
        has_b2=bool(np.any(b2)),
    )


def kernel(**inputs):
    from concourse.bass_utils import run_bass_kernel_spmd

    x = np.asarray(inputs["x"], np.float32)
    n = x.shape[0]
    npc = n // N_CORES
    w = _prep_weights(inputs)

    nc = build_nc(npc, has_qkv_bias=w["has_qkv_bias"], has_bo=w["has_bo"],
                  has_c2f=w["has_c2f"], has_b2=w["has_b2"])

    shared = dict(rwq=w["rwq"], rwk=w["rwk"], rwkw=w["rwkw"], rwv=w["rwv"],
                  rwo2=w["rwo2"], rw1=w["rw1"], w2t=w["w2t"], mask=w["mask"],
                  c2q=w["c2q"], c2k=w["c2k"], c2kw=w["c2kw"], c2v=w["c2v"],
                  bo=w["bo"], c2f=w["c2f"], b2=w["b2"])
    in_maps = []
    for core in range(N_CORES):
        m = dict(shared)
        m["x"] = np.ascontiguousarray(x[core * npc:(core + 1) * npc])
        in_maps.append(m)

    res = run_bass_kernel_spmd(nc, in_maps, list(range(N_CORES)))
    out = np.concatenate([np.asarray(res.results[c]["out"])
                          for c in range(N_CORES)], axis=0)
    return out.astype(np.float32)


# revision 24
# speedup vs baseline: 1.2142x; 1.2142x over previous
"""Trainium2 Bass kernel for EnhancedGraphTransformerLayer.

Layer: LN1 -> QKV proj -> per-node 8x8 head attention -> O proj -> residual
       -> LN2 -> FFN(512->2048->512, relu) -> residual.

Strategy (per NeuronCore, data-parallel over nodes, 8 cores):
- All big matmuls in bf16 on the PE (fp32 accumulate in PSUM), activations
  flow feature-transposed ([feature, node]) with weights stationary.
- Projections run at chunk granularity (512 nodes) with N=512 moving
  operands to keep the PE warm and amortize LDWEIGHTS.
- Per-node 8-head attention uses 16-node sub-group packing WITHOUT any
  data-movement gather: the (128,128) score matrix for a sub-group is
  assembled from 4 quadrant matmuls (K=64) whose operands are strided APs
  directly into the projection outputs. A host-side column-swapped copy of
  Wk ("rwkw") provides k-heads of either parity at either partition half so
  stationary/moving partition bases always match.
- A block-diagonal mask (multiplied on GpSimd) zeroes cross-node terms
  after exp; an appended ones-column of V yields softmax denominators
  inside the AV matmul.
- Attention-output transposes are split by head parity so the O projection
  contracts over K=128 (full PE array) in 4 matmuls.
- LayerNorm stats via bn_stats/bn_aggr in natural layout; gamma/beta are
  folded into weights/biases on the host.
"""

import numpy as np
import ml_dtypes
from contextlib import ExitStack

F8 = ml_dtypes.float8_e4m3fn

E = 512
H = 8
D = 64
F = 2048
EPS = 1e-5
N_NODES = 65536
N_CORES = 8
BF = ml_dtypes.bfloat16


def build_nc(npc, has_qkv_bias=False, has_bo=False, has_c2f=False,
             has_b2=False, fix_waits=True):
    import concourse.bass as bass
    import concourse.mybir as mybir

    f32 = mybir.dt.float32
    bf16 = mybir.dt.bfloat16
    f8 = mybir.dt.float8e4

    nc = bass.Bass()
    ins = dict(
        x=nc.dram_tensor("x", (npc, E), f32, kind="ExternalInput").ap(),
        rwq=nc.dram_tensor("rwq", (E, E), bf16, kind="ExternalInput").ap(),
        rwk=nc.dram_tensor("rwk", (E, E), bf16, kind="ExternalInput").ap(),
        rwkw=nc.dram_tensor("rwkw", (E, E), bf16, kind="ExternalInput").ap(),
        rwv=nc.dram_tensor("rwv", (E, E), bf16, kind="ExternalInput").ap(),
        rwo2=nc.dram_tensor("rwo2", (E, E), bf16, kind="ExternalInput").ap(),
        rw1=nc.dram_tensor("rw1", (E, F), f8, kind="ExternalInput").ap(),
        w2t=nc.dram_tensor("w2t", (F, E), f8, kind="ExternalInput").ap(),
        mask=nc.dram_tensor("mask", (128, 128), bf16, kind="ExternalInput").ap(),
        c2q=nc.dram_tensor("c2q", (E,), f32, kind="ExternalInput").ap(),
        c2k=nc.dram_tensor("c2k", (E,), f32, kind="ExternalInput").ap(),
        c2kw=nc.dram_tensor("c2kw", (E,), f32, kind="ExternalInput").ap(),
        c2v=nc.dram_tensor("c2v", (E,), f32, kind="ExternalInput").ap(),
        bo=nc.dram_tensor("bo", (E,), bf16, kind="ExternalInput").ap(),
        c2f=nc.dram_tensor("c2f", (F,), f32, kind="ExternalInput").ap(),
        b2=nc.dram_tensor("b2", (E,), f32, kind="ExternalInput").ap(),
    )
    out_ap = nc.dram_tensor("out", (npc, E), f32, kind="ExternalOutput").ap()
    build_body(nc, ins, out_ap, npc, has_qkv_bias=has_qkv_bias,
               has_bo=has_bo, has_c2f=has_c2f, has_b2=has_b2,
               fix_waits=fix_waits)
    return nc


def build_body(nc, ins, out_d, npc, has_qkv_bias=False, has_bo=False,
               has_c2f=False, has_b2=False, fix_waits=True):
    import concourse.bass as bass
    import concourse.mybir as mybir
    from concourse.tile import TileContext
    from concourse.masks import make_identity

    f32 = mybir.dt.float32
    bf16 = mybir.dt.bfloat16
    f8 = mybir.dt.float8e4
    AL = mybir.AluOpType
    AF = mybir.ActivationFunctionType
    DR = mybir.MatmulPerfMode.DoubleRow

    n_groups = npc // 128
    gpc = 4 if n_groups % 4 == 0 else 1  # groups per chunk
    n_chunks = n_groups // gpc
    npcch = 128 * gpc  # nodes per chunk
    nsb = npcch // 16  # 16-node sub-groups per chunk

    x_d = ins["x"]
    rwq_d, rwk_d, rwkw_d, rwv_d = ins["rwq"], ins["rwk"], ins["rwkw"], ins["rwv"]
    rwo2_d, rw1_d, w2t_d, mask_d = ins["rwo2"], ins["rw1"], ins["w2t"], ins["mask"]
    c2q_d, c2k_d, c2kw_d, c2v_d = ins["c2q"], ins["c2k"], ins["c2kw"], ins["c2v"]
    bo_d, c2f_d, b2_d = ins["bo"], ins["c2f"], ins["b2"]

    with TileContext(nc) as tc, ExitStack() as ctx:
        wpool = ctx.enter_context(tc.tile_pool(name="w", bufs=1))
        pool = ctx.enter_context(tc.tile_pool(name="act", bufs=1))
        psum = ctx.enter_context(tc.tile_pool(name="ps", bufs=1, space="PSUM"))

        # ---- constants / weights ----
        rwq_sb = wpool.tile([128, 4, E], bf16, tag="rwq")
        rwk_sb = wpool.tile([128, 4, E], bf16, tag="rwk")
        rwkw_sb = wpool.tile([128, 4, E], bf16, tag="rwkw")
        rwv_sb = wpool.tile([128, 4, E], bf16, tag="rwv")
        nc.sync.dma_start(out=rwq_sb, in_=rwq_d.rearrange("(t p) e -> p t e", p=128))
        nc.sync.dma_start(out=rwk_sb, in_=rwk_d.rearrange("(t p) e -> p t e", p=128))
        nc.sync.dma_start(out=rwkw_sb, in_=rwkw_d.rearrange("(t p) e -> p t e", p=128))
        nc.sync.dma_start(out=rwv_sb, in_=rwv_d.rearrange("(t p) e -> p t e", p=128))
        # rwo2[(h2*64+d), h1, fo] = Wo[fo, (2*h1+h2)*64+d]
        rwo2_sb = wpool.tile([128, 4, E], bf16, tag="rwo2")
        nc.scalar.dma_start(out=rwo2_sb, in_=rwo2_d.rearrange("(p t) e -> p t e", t=4))
        rw1_sb = wpool.tile([128, 4, F], f8, tag="rw1")
        nc.scalar.dma_start(out=rw1_sb, in_=rw1_d.rearrange("(t p) f -> p t f", p=128))
        w2t_sb = wpool.tile([128, 16, E], f8, tag="w2t")
        nc.scalar.dma_start(out=w2t_sb, in_=w2t_d.rearrange("(t p) e -> p t e", p=128))
        mask_sb = wpool.tile([128, 128], bf16, tag="mask")
        nc.sync.dma_start(out=mask_sb, in_=mask_d)
        ident128 = wpool.tile([128, 128], bf16, tag="id128")
        make_identity(nc, ident128)
        eps_sb = wpool.tile([128, 1], f32, tag="eps")
        nc.vector.memset(eps_sb, EPS)
        vaugA = wpool.tile([128, 4, 66], bf16, tag="vaugA")
        vaugB = wpool.tile([128, 4, 66], bf16, tag="vaugB")
        nc.vector.memset(vaugA[:, :, 64:65], 1.0)
        nc.vector.memset(vaugB[:, :, 64:65], 1.0)
        if has_qkv_bias:
            c2q_sb = wpool.tile([128, 4], f32, tag="c2q")
            c2k_sb = wpool.tile([128, 4], f32, tag="c2k")
            c2kw_sb = wpool.tile([128, 4], f32, tag="c2kw")
            c2v_sb = wpool.tile([128, 4], f32, tag="c2v")
            nc.sync.dma_start(out=c2q_sb, in_=c2q_d.rearrange("(t p) -> p t", p=128))
            nc.sync.dma_start(out=c2k_sb, in_=c2k_d.rearrange("(t p) -> p t", p=128))
            nc.sync.dma_start(out=c2kw_sb, in_=c2kw_d.rearrange("(t p) -> p t", p=128))
            nc.sync.dma_start(out=c2v_sb, in_=c2v_d.rearrange("(t p) -> p t", p=128))
        if has_bo:
            ones1_sb = wpool.tile([1, 128], bf16, tag="ones1")
            nc.vector.memset(ones1_sb, 1.0)
            bo_sb = wpool.tile([1, E], bf16, tag="bo")
            nc.sync.dma_start(out=bo_sb, in_=bo_d.rearrange("(o e) -> o e", o=1))
        if has_c2f:
            c2f_sb = wpool.tile([128, 16], f32, tag="c2f")
            nc.sync.dma_start(out=c2f_sb, in_=c2f_d.rearrange("(t p) -> p t", p=128))
        if has_b2:
            b2_sb = wpool.tile([128, 4], f32, tag="b2")
            nc.sync.dma_start(out=b2_sb, in_=b2_d.rearrange("(t p) -> p t", p=128))

        def apx(tile_ap, off, dims):
            """Custom AP into tile at flat-element offset `off`."""
            return bass.AP(tensor=tile_ap.tensor, offset=tile_ap.offset + off,
                           ap=[list(d) for d in dims])

        def bcast_u(small, n_u, n_d):
            """(128, n_u) AP broadcast to (128, n_u, n_d) via stride-0."""
            return bass.AP(tensor=small.tensor, offset=small.offset,
                           ap=[small.ap[0], [1, n_u], [0, n_d]])

        def layernorm_to_bf16(x_sb, tagp):
            stat = pool.tile([128, 6], f32, tag=tagp + "stat", bufs=2, name=tagp + "stat")
            nc.vector.bn_stats(out=stat, in_=x_sb)
            mv = pool.tile([128, 2], f32, tag=tagp + "mv", bufs=2, name=tagp + "mv")
            nc.vector.bn_aggr(out=mv, in_=stat)
            rs = pool.tile([128, 1], f32, tag=tagp + "rs", bufs=2, name=tagp + "rs")
            nc.scalar.activation(out=rs, in_=mv[:, 1:2], func=AF.Ln,
                                 bias=eps_sb, scale=1.0)
            nc.scalar.activation(out=rs, in_=rs, func=AF.Exp, scale=-0.5)
            zb = pool.tile([128, E], bf16, tag=tagp + "zb", bufs=2, name=tagp + "zb")
            nc.vector.tensor_scalar(out=zb, in0=x_sb, scalar1=mv[:, 0:1],
                                    scalar2=rs, op0=AL.subtract, op1=AL.mult)
            return zb

        QP = 4 * E      # per-partition element pitch of [128, 4, E] tiles

        def emit_ln1_group(c, gi):
            """x load + LN1 DVE chain for one group (no PE work)."""
            g = c * gpc + gi
            x_sb = pool.tile([128, E], f32, tag="x", bufs=2 * gpc,
                             name="x_sb")
            nc.sync.dma_start(out=x_sb, in_=x_d[g * 128:(g + 1) * 128, :])
            return x_sb, layernorm_to_bf16(x_sb, "ln1")

        def emit_ln1_loads(c):
            x_tiles, zb_tiles = [], []
            for gi in range(gpc):
                x_sb, zb = emit_ln1_group(c, gi)
                x_tiles.append(x_sb)
                zb_tiles.append(zb)
            return x_tiles, zb_tiles

        def emit_zbT(zb_tiles):
            """PE transposes of LN1 outputs into chunk-level zbT."""
            zbT_ch = pool.tile([128, 4, npcch], bf16, tag="zbT", bufs=2,
                               name="zbT_ch")
            for gi in range(gpc):
                zbT_ps = psum.tile([128, 4, 128], bf16, tag="tp", bufs=2,
                                   name="zbT_ps")
                for tau in range(4):
                    nc.tensor.transpose(zbT_ps[:, tau, :],
                                        zb_tiles[gi][:, 128 * tau:128 * (tau + 1)],
                                        ident128[:, :])
                dst = zbT_ch[:, :, gi * 128:(gi + 1) * 128]
                if gi % 2 == 0:
                    nc.vector.tensor_copy(out=dst, in_=zbT_ps)
                else:
                    nc.scalar.activation(out=dst, in_=zbT_ps, func=AF.Copy)
            return zbT_ch

        def emit_chunk(c, x_tiles, zbT_ch, fillers=(), next_c=None):
            # ---- Q/K/Kw/V projections (feature-transposed, chunk-wide) ----
            # SBUF layout [128, nsb, 64]: col (sb, tau*16+j) so each
            # sub-group's matmul operand is one contiguous 64-col run
            # (matmul moving APs allow only one free dimension).
            qTb = pool.tile([128, nsb, 64], bf16, tag="qTb", bufs=2, name="qTb")
            kTb = pool.tile([128, nsb, 64], bf16, tag="kTb", bufs=2, name="kTb")
            kTw = pool.tile([128, nsb, 64], bf16, tag="kTw", bufs=2, name="kTw")
            vTb = pool.tile([128, nsb, 64], bf16, tag="vTb", bufs=2, name="vTb")
            projs = [(rwq_sb, qTb, "c2q", 0), (rwk_sb, kTb, "c2k", 1),
                     (rwkw_sb, kTw, "c2kw", 0), (rwv_sb, vTb, "c2v", 1)]
            if has_qkv_bias:
                cmap = dict(c2q=c2q_sb, c2k=c2k_sb, c2kw=c2kw_sb, c2v=c2v_sb)
            for tau in range(4):
                for rw_sb, dst, cname, on_scalar in projs:
                    pp = psum.tile([128, npcch], f32, tag="big", bufs=3, name="pp")
                    for et in range(4):
                        nc.tensor.matmul(pp,
                                         rw_sb[:, et, 128 * tau:128 * (tau + 1)],
                                         zbT_ch[:, et, :],
                                         start=(et == 0), stop=(et == 3))
                    # dst cols (sb: stride 64) x (j: 16) at offset tau*16
                    dst_ap = apx(dst, tau * 16,
                                 [[nsb * 64, 128], [64, nsb], [1, 16]])
                    if has_qkv_bias:
                        cs = cmap[cname]
                        if on_scalar:
                            nc.scalar.activation(out=dst_ap, in_=pp,
                                                 func=AF.Identity,
                                                 bias=cs[:, tau:tau + 1])
                        else:
                            nc.vector.tensor_scalar_add(out=dst_ap,
                                                        in0=pp,
                                                        scalar1=cs[:, tau:tau + 1])
                    elif on_scalar:
                        nc.scalar.activation(out=dst_ap, in_=pp,
                                             func=AF.Copy)
                    else:
                        nc.vector.tensor_copy(out=dst_ap, in_=pp)

            # ---- attention per group (two-phase emission keeps PE fed) ----
            x2_tiles = []
            nxt = []
            z2bT_ch = pool.tile([128, 4, npcch], f8, tag="z2bT",
                                bufs=2, name="z2bT_ch")
            mask_b = bass.AP(tensor=mask_sb.tensor, offset=mask_sb.offset,
                             ap=[list(mask_sb.ap[0]), [0, 4], [1, 128]])
            pend_z2b = None  # (z2b, gi) transposed one group later

            def emit_z2bT(z2b, gi):
                z2bT_ps = psum.tile([128, 4, 128], bf16, tag="tp", bufs=2,
                                    name="z2bT_ps")
                for tau in range(4):
                    nc.tensor.transpose(z2bT_ps[:, tau, :],
                                        z2b[:, 128 * tau:128 * (tau + 1)],
                                        ident128[:, :])
                nc.scalar.activation(out=z2bT_ch[:, :, gi * 128:(gi + 1) * 128],
                                     in_=z2bT_ps, func=AF.Copy)

            for gi in range(gpc):
                pg2 = pool.tile([128, 4, 128], bf16, tag="pg2", bufs=2,
                                name="pg2")
                halves = []
                for half in range(2):
                    sb0 = gi * 8 + half * 4
                    # scores: 4 quadrant matmuls per sub-group. Split into
                    # two PSUM tiles by column half (h2) so concurrent MMs
                    # on different PE row-groups never drain to the same
                    # PSUM partitions of one bank (HW write collision).
                    s2L = psum.tile([128, 4, 64], f32, tag="att", bufs=3,
                                    name="s2L")
                    s2R = psum.tile([128, 4, 64], f32, tag="att", bufs=3,
                                    name="s2R")
                    for u in range(4):
                        sb = sb0 + u
                        for g2, h2 in ((0, 0), (1, 1), (1, 0), (0, 1)):
                            kX = kTb if g2 == h2 else kTw
                            off = h2 * 64 * QP + sb * 64
                            stat = apx(kX, off, [[QP, 64], [1, 64]])
                            mov = apx(qTb, off, [[QP, 64], [1, 64]])
                            s2 = s2L if h2 == 0 else s2R
                            nc.tensor.matmul(
                                s2[g2 * 64:(g2 + 1) * 64, u, :],
                                stat, mov, start=True, stop=True)
                    e_sb = pool.tile([128, 4, 128], bf16, tag="esb", bufs=2,
                                     name="e_sb")
                    eL = apx(e_sb, 0, [[512, 128], [128, 4], [1, 64]])
                    eR = apx(e_sb, 64, [[512, 128], [128, 4], [1, 64]])
                    nc.scalar.activation(out=eL, in_=s2L, func=AF.Exp)
                    nc.scalar.activation(out=eR, in_=s2R, func=AF.Exp)
                    a_sb = pool.tile([128, 4, 128], bf16, tag="asb", bufs=2,
                                     name="a_sb")
                    nc.gpsimd.tensor_tensor(out=a_sb, in0=e_sb, in1=mask_b,
                                            op=AL.mult)
                    # v sub-group packing via per-half transposes
                    vt_ps = psum.tile([128, 4, 64], bf16, tag="tp", bufs=2,
                                      name="vt_ps")
                    for u in range(4):
                        sb = sb0 + u
                        for g2 in range(2):
                            src = apx(vTb, g2 * 64 * QP + sb * 64,
                                      [[QP, 64], [1, 64]])
                            nc.tensor.transpose(
                                vt_ps[g2 * 64:(g2 + 1) * 64, u, :], src,
                                ident128[g2 * 64:(g2 + 1) * 64,
                                         g2 * 64:(g2 + 1) * 64])
                    vaug = vaugA if half == 0 else vaugB
                    nc.vector.tensor_copy(out=vaug[:, :, 0:64], in_=vt_ps)
                    halves.append((a_sb, vaug))

                for half in range(2):
                    a_sb, vaug = halves[half]
                    # AV (+denominator)
                    outS = psum.tile([128, 4, 66], f32, tag="att", bufs=3,
                                     name="outS")
                    for u in range(4):
                        nc.tensor.matmul(outS[:, u, 0:65], a_sb[:, u, :],
                                         vaug[:, u, 0:65], start=True,
                                         stop=True)
                    # normalize
                    recip = pool.tile([128, 4], f32, tag="recip", bufs=2,
                                      name="recip")
                    nc.vector.reciprocal(out=recip, in_=outS[:, :, 64])
                    ogb = pool.tile([128, 4, 64], bf16, tag="ogb", bufs=2,
                                    name="ogb")
                    nc.vector.tensor_tensor(out=ogb, in0=outS[:, :, 0:64],
                                            in1=bcast_u(recip, 4, 64),
                                            op=AL.mult)
                    # out transposes split by head parity
                    # -> pg2[(h2,d), h1, (sb,j)]
                    p_ps = psum.tile([128, 4, 64], bf16, tag="tp", bufs=2,
                                     name="p_ps")
                    for u in range(4):
                        for h2 in range(2):
                            nc.tensor.transpose(
                                p_ps[h2 * 64:(h2 + 1) * 64, u, :],
                                ogb[h2 * 64:(h2 + 1) * 64, u, :],
                                ident128[h2 * 64:(h2 + 1) * 64,
                                         h2 * 64:(h2 + 1) * 64])
                    # pg2 col = h1*128 + sb*16 + j (per-h1 contiguous for
                    # the single-free-dim O-proj stationary)
                    pg2_ap = apx(pg2, half * 64,
                                 [[512, 128], [16, 4], [128, 4], [1, 16]])
                    nc.scalar.activation(out=pg2_ap, in_=p_ps, func=AF.Copy)

                # ---- O projection: K=128 over (h2, d), accumulate over h1 ----
                oproj_ps = psum.tile([128, E], f32, tag="big", bufs=3,
                                     name="oproj_ps")
                for h1 in range(4):
                    nc.tensor.matmul(oproj_ps, pg2[:, h1, :], rwo2_sb[:, h1, :],
                                     start=(h1 == 0),
                                     stop=(h1 == 3 and not has_bo))
                if has_bo:
                    nc.tensor.matmul(oproj_ps, ones1_sb, bo_sb,
                                     start=False, stop=True)

                # ---- residual 1 + LN2 (z2bT transposes deferred a group) ----
                x2_sb = pool.tile([128, E], f32, tag="x2", bufs=2 * gpc + 2,
                                  name="x2_sb")
                nc.vector.tensor_add(out=x2_sb, in0=x_tiles[gi], in1=oproj_ps)
                x2_tiles.append(x2_sb)
                z2b = layernorm_to_bf16(x2_sb, "ln2")
                if pend_z2b is not None:
                    emit_z2bT(*pend_z2b)
                pend_z2b = (z2b, gi)
                # next chunk's x load + LN1 chain, spread across groups
                if next_c is not None:
                    nxt.append(emit_ln1_group(next_c, gi))
                # dense-MM filler (prev chunk's FFN slice) keeps the PE's
                # HAM clock warm through the transpose-heavy attention
                if gi < len(fillers):
                    fillers[gi]()

            if len(fillers) > 4:
                fillers[4]()
                emit_z2bT(*pend_z2b)
                fillers[5]()
            else:
                emit_z2bT(*pend_z2b)
            return x2_tiles, z2bT_ch, nxt

        def make_ffn_pieces(c, z2bT_ch, x2_tiles):
            """FFN for chunk c as 6 closures: 4x (u1 ft block), u2, tail."""
            cell = {}

            def u1_block(ft0):
                def go():
                    if ft0 == 0:
                        cell["rT"] = pool.tile([128, 16, npcch], f8,
                                               tag="rt", bufs=1, name="rT_sb")
                    rT_sb = cell["rT"]
                    for ft in range(ft0, ft0 + 4):
                        u1_ps = psum.tile([128, npcch], f32, tag="big",
                                          bufs=3, name="u1_ps")
                        for t2 in range(2):
                            stat = apx(rw1_sb, t2 * 2 * F + ft * 128,
                                       [[4 * F, 128], [F, 2], [1, 128]])
                            mov = apx(z2bT_ch, t2 * 2 * npcch,
                                      [[4 * npcch, 128], [npcch, 2],
                                       [1, npcch]])
                            nc.tensor.matmul(u1_ps, stat, mov, perf_mode=DR,
                                             start=(t2 == 0), stop=(t2 == 1))
                        if has_c2f:
                            nc.vector.tensor_scalar(
                                out=rT_sb[:, ft, :], in0=u1_ps,
                                scalar1=c2f_sb[:, ft:ft + 1],
                                scalar2=0.0, op0=AL.add, op1=AL.max)
                        elif ft % 2 == 0:
                            nc.scalar.activation(out=rT_sb[:, ft, :],
                                                 in_=u1_ps, func=AF.Relu)
                        else:
                            nc.vector.tensor_scalar_max(out=rT_sb[:, ft, :],
                                                        in0=u1_ps, scalar1=0.0)
                return go

            def u2_all():
                rT_sb = cell["rT"]
                u2b_sb = pool.tile([128, 4, npcch], bf16, tag="u2b", bufs=2,
                                   name="u2b_sb")
                cell["u2b"] = u2b_sb
                for et in range(4):
                    u2_ps = psum.tile([128, npcch], f32, tag="big", bufs=3,
                                      name="u2_ps")
                    for t2 in range(8):
                        stat = apx(w2t_sb, t2 * 2 * E + et * 128,
                                   [[16 * E, 128], [E, 2], [1, 128]])
                        mov = apx(rT_sb, t2 * 2 * npcch,
                                  [[16 * npcch, 128], [npcch, 2], [1, npcch]])
                        nc.tensor.matmul(u2_ps, stat, mov, perf_mode=DR,
                                         start=(t2 == 0), stop=(t2 == 7))
                    if has_b2:
                        nc.vector.tensor_scalar_add(
                            out=u2b_sb[:, et, :], in0=u2_ps,
                            scalar1=b2_sb[:, et:et + 1])
                    elif et % 2 == 0:
                        nc.scalar.activation(out=u2b_sb[:, et, :], in_=u2_ps,
                                             func=AF.Copy)
                    else:
                        nc.vector.tensor_copy(out=u2b_sb[:, et, :], in_=u2_ps)

            def tail():
                u2b_sb = cell["u2b"]
                for gi in range(gpc):
                    g = c * gpc + gi
                    u2n_ps = psum.tile([128, 4, 128], bf16, tag="tp", bufs=2,
                                       name="u2n_ps")
                    for et in range(4):
                        nc.tensor.transpose(
                            u2n_ps[:, et, :],
                            u2b_sb[:, et, 128 * gi:128 * (gi + 1)],
                            ident128[:, :])
                    u2nat = pool.tile([128, 4, 128], bf16, tag="u2nat",
                                      bufs=2, name="u2nat")
                    if gi % 2 == 0:
                        nc.vector.tensor_copy(out=u2nat, in_=u2n_ps)
                    else:
                        nc.scalar.activation(out=u2nat, in_=u2n_ps,
                                             func=AF.Copy)
                    out_sb = pool.tile([128, E], f32, tag="osb", bufs=3,
                                       name="out_sb")
                    nc.vector.tensor_add(out=out_sb, in0=x2_tiles[gi],
                                         in1=u2nat)
                    nc.sync.dma_start(out=out_d[g * 128:(g + 1) * 128, :],
                                      in_=out_sb)

            return [u1_block(0), u1_block(4), u1_block(8), u1_block(12),
                    u2_all, tail]

        # ---- software-pipelined main loop (chunk-skewed, interleaved FFN) --
        x_next, zb_next = emit_ln1_loads(0)
        zbT_next = emit_zbT(zb_next)
        fillers = ()
        for c in range(n_chunks):
            x_cur, zbT_cur = x_next, zbT_next
            nc_arg = c + 1 if c + 1 < n_chunks else None
            x2_tiles, z2bT_ch, nxt = emit_chunk(c, x_cur, zbT_cur, fillers,
                                                next_c=nc_arg)
            if nc_arg is not None:
                x_next = [t[0] for t in nxt]
                zbT_next = emit_zbT([t[1] for t in nxt])
            fillers = make_ffn_pieces(c, z2bT_ch, x2_tiles)
        for piece in fillers:
            piece()

    if fix_waits:
        _fix_sync_waits(nc)


def _fix_sync_waits(nc):
    """walrus limits inline sync waits to 1 per instruction. Tile can
    emit more. Split the excess into standalone InstEventSemaphore
    wait-carriers inserted immediately before the overweight instruction
    on the same engine - semantically identical (the waits still execute
    right before the instruction, in order)."""
    import concourse.mybir as mybir
    n = 0
    for f in nc.m.functions:
        for blk in f.blocks:
            insts = blk.instructions
            out = []
            dirty = False
            for inst in insts:
                si = inst.sync_info
                waits = list(si.on_wait) if (si and si.on_wait) else []
                limit = 1
                if len(waits) > limit:
                    ups = list(si.on_update) if (si and si.on_update) else []
                    up_ids = {u.id for u in ups}
                    # keep own-queue credit waits inline (DMA flow control)
                    waits.sort(key=lambda w: 0 if w.id in up_ids else 1)
                    keep, move = waits[:limit], waits[limit:]
                    for w in move:
                        n += 1
                        car = mybir.InstEventSemaphore(
                            name="WSPLIT-%d" % n, ins=[], outs=[])
                        car.engine = inst.engine
                        car.sync_info = mybir.SyncInfo(on_wait=[w],
                                                       on_update=[])
                        out.append(car)
                    inst.sync_info = mybir.SyncInfo(on_wait=keep,
                                                   on_update=ups)
                    dirty = True
                out.append(inst)
            if dirty:
                blk.instructions = out
    return n


def _prep_weights(inputs):
    """Host-side weight folding. Returns dict of np arrays + flags."""
    f32 = np.float32
    g1 = np.asarray(inputs["g1"], f32)
    beta1 = np.asarray(inputs["beta1"], f32)
    g2 = np.asarray(inputs["g2"], f32)
    beta2 = np.asarray(inputs["beta2"], f32)
    Wq = np.asarray(inputs["Wq"], f32)
    Wk = np.asarray(inputs["Wk"], f32)
    Wv = np.asarray(inputs["Wv"], f32)
    Wo = np.asarray(inputs["Wo"], f32)
    W1 = np.asarray(inputs["W1"], f32)
    W2 = np.asarray(inputs["W2"], f32)
    scale = np.float32(1.0 / np.sqrt(D))

    rwq = (Wq.T * g1[:, None] * scale).astype(BF)
    rwk = (Wk.T * g1[:, None]).astype(BF)
    rwv = (Wv.T * g1[:, None]).astype(BF)
    rw1 = (W1.T * g2[:, None]).astype(F8)
    w2t = W2.T.astype(F8)

    # column-swapped K weights: within each 128-col block, swap 64-halves
    sw = np.arange(E)
    sw = (sw // 128) * 128 + (sw % 128 + 64) % 128
    rwkw = np.ascontiguousarray(rwk[:, sw])

    # rwo2[(h2*64+d), h1, fo] = Wo[fo, (2*h1+h2)*64+d]  (stored [E, E] with
    # row index (h2*64+d)*4... laid out as [(p t) e] with p=(h2*64+d), t=h1)
    WoT = Wo.T.astype(BF)  # [E(h*64+d), E(fo)]
    rwo2 = np.zeros((E, E), BF)
    for h2 in range(2):
        for h1 in range(4):
            h = 2 * h1 + h2
            # destination rows (h2*64+d) at t=h1: flat row = (h2*64+d)*4 + h1
            rwo2[(np.arange(64) + h2 * 64) * 4 + h1, :] = WoT[h * 64:(h + 1) * 64, :]

    c2q = ((Wq @ beta1 + np.asarray(inputs["bq"], f32)) * scale).astype(f32)
    c2k = (Wk @ beta1 + np.asarray(inputs["bk"], f32)).astype(f32)
    c2kw = np.ascontiguousarray(c2k[sw])
    c2v = (Wv @ beta1 + np.asarray(inputs["bv"], f32)).astype(f32)
    bo = np.asarray(inputs["bo"], f32)
    c2f = (W1 @ beta2 + np.asarray(inputs["b1"], f32)).astype(f32)
    b2 = np.asarray(inputs["b2"], f32)

    mask = np.zeros((128, 128), f32)
    for i in range(16):
        for gg in range(8):
            for hh in range(8):
                mask[gg * 16 + i, hh * 16 + i] = 1.0

    return dict(
        rwq=rwq, rwk=rwk, rwkw=rwkw, rwv=rwv, rwo2=rwo2, rw1=rw1, w2t=w2t,
        mask=mask.astype(BF),
        c2q=c2q, c2k=c2k, c2kw=c2kw, c2v=c2v, bo=bo.astype(BF), c2f=c2f, b2=b2,
        has_qkv_bias=bool(np.any(c2q) or np.any(c2k) or np.any(c2v)),
        has_bo=bool(np.any(bo)), has_c2f=bool(np.any(c2f)),
        has_b2=bool(np.any(b2)),
    )


def kernel(**inputs):
    from concourse.bass_utils import run_bass_kernel_spmd

    x = np.asarray(inputs["x"], np.float32)
    n = x.shape[0]
    npc = n // N_CORES
    w = _prep_weights(inputs)

    nc = build_nc(npc, has_qkv_bias=w["has_qkv_bias"], has_bo=w["has_bo"],
                  has_c2f=w["has_c2f"], has_b2=w["has_b2"])

    shared = dict(rwq=w["rwq"], rwk=w["rwk"], rwkw=w["rwkw"], rwv=w["rwv"],
                  rwo2=w["rwo2"], rw1=w["rw1"], w2t=w["w2t"], mask=w["mask"],
                  c2q=w["c2q"], c2k=w["c2k"], c2kw=w["c2kw"], c2v=w["c2v"],
                  bo=w["bo"], c2f=w["c2f"], b2=w["b2"])
    in_maps = []
    for core in range(N_CORES):
        m = dict(shared)
        m["x"] = np.ascontiguousarray(x[core * npc:(core + 1) * npc])
        in_maps.append(m)

    res = run_bass_kernel_spmd(nc, in_maps, list(range(N_CORES)))
    out = np.concatenate([np.asarray(res.results[c]["out"])
                          for c in range(N_CORES)], axis=0)
    return out.astype(np.float32)


# revision 25
# speedup vs baseline: 1.2361x; 1.0180x over previous
"""Trainium2 Bass kernel for EnhancedGraphTransformerLayer.

Layer: LN1 -> QKV proj -> per-node 8x8 head attention -> O proj -> residual
       -> LN2 -> FFN(512->2048->512, relu) -> residual.

Strategy (per NeuronCore, data-parallel over nodes, 8 cores):
- All big matmuls in bf16 on the PE (fp32 accumulate in PSUM), activations
  flow feature-transposed ([feature, node]) with weights stationary.
- Projections run at chunk granularity (512 nodes) with N=512 moving
  operands to keep the PE warm and amortize LDWEIGHTS.
- Per-node 8-head attention uses 16-node sub-group packing WITHOUT any
  data-movement gather: the (128,128) score matrix for a sub-group is
  assembled from 4 quadrant matmuls (K=64) whose operands are strided APs
  directly into the projection outputs. A host-side column-swapped copy of
  Wk ("rwkw") provides k-heads of either parity at either partition half so
  stationary/moving partition bases always match.
- A block-diagonal mask (multiplied on GpSimd) zeroes cross-node terms
  after exp; an appended ones-column of V yields softmax denominators
  inside the AV matmul.
- Attention-output transposes are split by head parity so the O projection
  contracts over K=128 (full PE array) in 4 matmuls.
- LayerNorm stats via bn_stats/bn_aggr in natural layout; gamma/beta are
  folded into weights/biases on the host.
"""

import numpy as np
import ml_dtypes
from contextlib import ExitStack

F8 = ml_dtypes.float8_e4m3fn

E = 512
H = 8
D = 64
F = 2048
EPS = 1e-5
N_NODES = 65536
N_CORES = 8
BF = ml_dtypes.bfloat16


def build_nc(npc, has_qkv_bias=False, has_bo=False, has_c2f=False,
             has_b2=False, fix_waits=True):
    import concourse.bass as bass
    import concourse.mybir as mybir

    f32 = mybir.dt.float32
    bf16 = mybir.dt.bfloat16
    f8 = mybir.dt.float8e4

    nc = bass.Bass()
    ins = dict(
        x=nc.dram_tensor("x", (npc, E), f32, kind="ExternalInput").ap(),
        rwq=nc.dram_tensor("rwq", (E, E), bf16, kind="ExternalInput").ap(),
        rwk=nc.dram_tensor("rwk", (E, E), bf16, kind="ExternalInput").ap(),
        rwkw=nc.dram_tensor("rwkw", (E, E), bf16, kind="ExternalInput").ap(),
        rwv=nc.dram_tensor("rwv", (E, E), bf16, kind="ExternalInput").ap(),
        rwo2=nc.dram_tensor("rwo2", (E, E), bf16, kind="ExternalInput").ap(),
        rw1=nc.dram_tensor("rw1", (E, F), f8, kind="ExternalInput").ap(),
        w2t=nc.dram_tensor("w2t", (F, E), f8, kind="ExternalInput").ap(),
        mask=nc.dram_tensor("mask", (128, 128), bf16, kind="ExternalInput").ap(),
        c2q=nc.dram_tensor("c2q", (E,), f32, kind="ExternalInput").ap(),
        c2k=nc.dram_tensor("c2k", (E,), f32, kind="ExternalInput").ap(),
        c2kw=nc.dram_tensor("c2kw", (E,), f32, kind="ExternalInput").ap(),
        c2v=nc.dram_tensor("c2v", (E,), f32, kind="ExternalInput").ap(),
        bo=nc.dram_tensor("bo", (E,), bf16, kind="ExternalInput").ap(),
        c2f=nc.dram_tensor("c2f", (F,), f32, kind="ExternalInput").ap(),
        b2=nc.dram_tensor("b2", (E,), f32, kind="ExternalInput").ap(),
    )
    out_ap = nc.dram_tensor("out", (npc, E), f32, kind="ExternalOutput").ap()
    build_body(nc, ins, out_ap, npc, has_qkv_bias=has_qkv_bias,
               has_bo=has_bo, has_c2f=has_c2f, has_b2=has_b2,
               fix_waits=fix_waits)
    return nc


def build_body(nc, ins, out_d, npc, has_qkv_bias=False, has_bo=False,
               has_c2f=False, has_b2=False, fix_waits=True):
    import concourse.bass as bass
    import concourse.mybir as mybir
    from concourse.tile import TileContext
    from concourse.masks import make_identity

    f32 = mybir.dt.float32
    bf16 = mybir.dt.bfloat16
    f8 = mybir.dt.float8e4
    AL = mybir.AluOpType
    AF = mybir.ActivationFunctionType
    DR = mybir.MatmulPerfMode.DoubleRow

    n_groups = npc // 128
    gpc = 4 if n_groups % 4 == 0 else 1  # groups per chunk
    n_chunks = n_groups // gpc
    npcch = 128 * gpc  # nodes per chunk
    nsb = npcch // 16  # 16-node sub-groups per chunk

    x_d = ins["x"]
    rwq_d, rwk_d, rwkw_d, rwv_d = ins["rwq"], ins["rwk"], ins["rwkw"], ins["rwv"]
    rwo2_d, rw1_d, w2t_d, mask_d = ins["rwo2"], ins["rw1"], ins["w2t"], ins["mask"]
    c2q_d, c2k_d, c2kw_d, c2v_d = ins["c2q"], ins["c2k"], ins["c2kw"], ins["c2v"]
    bo_d, c2f_d, b2_d = ins["bo"], ins["c2f"], ins["b2"]

    with TileContext(nc) as tc, ExitStack() as ctx:
        wpool = ctx.enter_context(tc.tile_pool(name="w", bufs=1))
        pool = ctx.enter_context(tc.tile_pool(name="act", bufs=1))
        psum = ctx.enter_context(tc.tile_pool(name="ps", bufs=1, space="PSUM"))

        # ---- constants / weights ----
        rwq_sb = wpool.tile([128, 4, E], bf16, tag="rwq")
        rwk_sb = wpool.tile([128, 4, E], bf16, tag="rwk")
        rwkw_sb = wpool.tile([128, 4, E], bf16, tag="rwkw")
        rwv_sb = wpool.tile([128, 4, E], bf16, tag="rwv")
        nc.sync.dma_start(out=rwq_sb, in_=rwq_d.rearrange("(t p) e -> p t e", p=128))
        nc.sync.dma_start(out=rwk_sb, in_=rwk_d.rearrange("(t p) e -> p t e", p=128))
        nc.sync.dma_start(out=rwkw_sb, in_=rwkw_d.rearrange("(t p) e -> p t e", p=128))
        nc.sync.dma_start(out=rwv_sb, in_=rwv_d.rearrange("(t p) e -> p t e", p=128))
        # rwo2[(h2*64+d), h1, fo] = Wo[fo, (2*h1+h2)*64+d]
        rwo2_sb = wpool.tile([128, 4, E], bf16, tag="rwo2")
        nc.scalar.dma_start(out=rwo2_sb, in_=rwo2_d.rearrange("(p t) e -> p t e", t=4))
        rw1_sb = wpool.tile([128, 4, F], f8, tag="rw1")
        nc.scalar.dma_start(out=rw1_sb, in_=rw1_d.rearrange("(t p) f -> p t f", p=128))
        w2t_sb = wpool.tile([128, 16, E], f8, tag="w2t")
        nc.scalar.dma_start(out=w2t_sb, in_=w2t_d.rearrange("(t p) e -> p t e", p=128))
        mask_sb = wpool.tile([128, 128], bf16, tag="mask")
        nc.sync.dma_start(out=mask_sb, in_=mask_d)
        ident128 = wpool.tile([128, 128], bf16, tag="id128")
        make_identity(nc, ident128)
        eps_sb = wpool.tile([128, 1], f32, tag="eps")
        nc.vector.memset(eps_sb, EPS)
        vaugA = wpool.tile([128, 4, 66], bf16, tag="vaugA")
        vaugB = wpool.tile([128, 4, 66], bf16, tag="vaugB")
        nc.vector.memset(vaugA[:, :, 64:65], 1.0)
        nc.vector.memset(vaugB[:, :, 64:65], 1.0)
        if has_qkv_bias:
            c2q_sb = wpool.tile([128, 4], f32, tag="c2q")
            c2k_sb = wpool.tile([128, 4], f32, tag="c2k")
            c2kw_sb = wpool.tile([128, 4], f32, tag="c2kw")
            c2v_sb = wpool.tile([128, 4], f32, tag="c2v")
            nc.sync.dma_start(out=c2q_sb, in_=c2q_d.rearrange("(t p) -> p t", p=128))
            nc.sync.dma_start(out=c2k_sb, in_=c2k_d.rearrange("(t p) -> p t", p=128))
            nc.sync.dma_start(out=c2kw_sb, in_=c2kw_d.rearrange("(t p) -> p t", p=128))
            nc.sync.dma_start(out=c2v_sb, in_=c2v_d.rearrange("(t p) -> p t", p=128))
        if has_bo:
            ones1_sb = wpool.tile([1, 128], bf16, tag="ones1")
            nc.vector.memset(ones1_sb, 1.0)
            bo_sb = wpool.tile([1, E], bf16, tag="bo")
            nc.sync.dma_start(out=bo_sb, in_=bo_d.rearrange("(o e) -> o e", o=1))
        if has_c2f:
            c2f_sb = wpool.tile([128, 16], f32, tag="c2f")
            nc.sync.dma_start(out=c2f_sb, in_=c2f_d.rearrange("(t p) -> p t", p=128))
        if has_b2:
            b2_sb = wpool.tile([128, 4], f32, tag="b2")
            nc.sync.dma_start(out=b2_sb, in_=b2_d.rearrange("(t p) -> p t", p=128))

        def apx(tile_ap, off, dims):
            """Custom AP into tile at flat-element offset `off`."""
            return bass.AP(tensor=tile_ap.tensor, offset=tile_ap.offset + off,
                           ap=[list(d) for d in dims])

        def bcast_u(small, n_u, n_d):
            """(128, n_u) AP broadcast to (128, n_u, n_d) via stride-0."""
            return bass.AP(tensor=small.tensor, offset=small.offset,
                           ap=[small.ap[0], [1, n_u], [0, n_d]])

        def layernorm_to_bf16(x_sb, tagp):
            stat = pool.tile([128, 6], f32, tag=tagp + "stat", bufs=2, name=tagp + "stat")
            nc.vector.bn_stats(out=stat, in_=x_sb)
            mv = pool.tile([128, 2], f32, tag=tagp + "mv", bufs=2, name=tagp + "mv")
            nc.vector.bn_aggr(out=mv, in_=stat)
            rs = pool.tile([128, 1], f32, tag=tagp + "rs", bufs=2, name=tagp + "rs")
            nc.scalar.activation(out=rs, in_=mv[:, 1:2], func=AF.Ln,
                                 bias=eps_sb, scale=1.0)
            nc.scalar.activation(out=rs, in_=rs, func=AF.Exp, scale=-0.5)
            zb = pool.tile([128, E], bf16, tag=tagp + "zb", bufs=2, name=tagp + "zb")
            nc.vector.tensor_scalar(out=zb, in0=x_sb, scalar1=mv[:, 0:1],
                                    scalar2=rs, op0=AL.subtract, op1=AL.mult)
            return zb

        QP = 4 * E      # per-partition element pitch of [128, 4, E] tiles

        def emit_ln1_group(c, gi):
            """x load + LN1 DVE chain for one group (no PE work)."""
            g = c * gpc + gi
            x_sb = pool.tile([128, E], f32, tag="x", bufs=2 * gpc,
                             name="x_sb")
            nc.sync.dma_start(out=x_sb, in_=x_d[g * 128:(g + 1) * 128, :])
            return x_sb, layernorm_to_bf16(x_sb, "ln1")

        def emit_ln1_loads(c):
            x_tiles, zb_tiles = [], []
            for gi in range(gpc):
                x_sb, zb = emit_ln1_group(c, gi)
                x_tiles.append(x_sb)
                zb_tiles.append(zb)
            return x_tiles, zb_tiles

        def emit_zbT(zb_tiles):
            """PE transposes of LN1 outputs into chunk-level zbT."""
            zbT_ch = pool.tile([128, 4, npcch], bf16, tag="zbT", bufs=2,
                               name="zbT_ch")
            for gi in range(gpc):
                zbT_ps = psum.tile([128, 4, 128], bf16, tag="tp", bufs=2,
                                   name="zbT_ps")
                for tau in range(4):
                    nc.tensor.transpose(zbT_ps[:, tau, :],
                                        zb_tiles[gi][:, 128 * tau:128 * (tau + 1)],
                                        ident128[:, :])
                dst = zbT_ch[:, :, gi * 128:(gi + 1) * 128]
                if gi % 2 == 0:
                    nc.vector.tensor_copy(out=dst, in_=zbT_ps)
                else:
                    nc.scalar.activation(out=dst, in_=zbT_ps, func=AF.Copy)
            return zbT_ch

        def emit_chunk(c, x_tiles, zbT_ch, fillers=(), next_c=None):
            # ---- Q/K/Kw/V projections (feature-transposed, chunk-wide) ----
            # SBUF layout [128, nsb, 64]: col (sb, tau*16+j) so each
            # sub-group's matmul operand is one contiguous 64-col run
            # (matmul moving APs allow only one free dimension).
            qTb = pool.tile([128, nsb, 64], bf16, tag="qTb", bufs=2, name="qTb")
            kTb = pool.tile([128, nsb, 64], bf16, tag="kTb", bufs=2, name="kTb")
            kTw = pool.tile([128, nsb, 64], bf16, tag="kTw", bufs=2, name="kTw")
            vTb = pool.tile([128, nsb, 64], bf16, tag="vTb", bufs=2, name="vTb")
            projs = [(rwq_sb, qTb, "c2q", 0), (rwk_sb, kTb, "c2k", 1),
                     (rwkw_sb, kTw, "c2kw", 0), (rwv_sb, vTb, "c2v", 1)]
            if has_qkv_bias:
                cmap = dict(c2q=c2q_sb, c2k=c2k_sb, c2kw=c2kw_sb, c2v=c2v_sb)
            for tau in range(4):
                for rw_sb, dst, cname, on_scalar in projs:
                    pp = psum.tile([128, npcch], f32, tag="big", bufs=3, name="pp")
                    for et in range(4):
                        nc.tensor.matmul(pp,
                                         rw_sb[:, et, 128 * tau:128 * (tau + 1)],
                                         zbT_ch[:, et, :],
                                         start=(et == 0), stop=(et == 3))
                    # dst cols (sb: stride 64) x (j: 16) at offset tau*16
                    dst_ap = apx(dst, tau * 16,
                                 [[nsb * 64, 128], [64, nsb], [1, 16]])
                    if has_qkv_bias:
                        cs = cmap[cname]
                        if on_scalar:
                            nc.scalar.activation(out=dst_ap, in_=pp,
                                                 func=AF.Identity,
                                                 bias=cs[:, tau:tau + 1])
                        else:
                            nc.vector.tensor_scalar_add(out=dst_ap,
                                                        in0=pp,
                                                        scalar1=cs[:, tau:tau + 1])
                    elif on_scalar:
                        nc.scalar.activation(out=dst_ap, in_=pp,
                                             func=AF.Copy)
                    else:
                        nc.vector.tensor_copy(out=dst_ap, in_=pp)

            # ---- attention per group (two-phase emission keeps PE fed) ----
            x2_tiles = []
            nxt = []
            fill_i = [0]
            z2bT_ch = pool.tile([128, 4, npcch], f8, tag="z2bT",
                                bufs=2, name="z2bT_ch")
            mask_b = bass.AP(tensor=mask_sb.tensor, offset=mask_sb.offset,
                             ap=[list(mask_sb.ap[0]), [0, 4], [1, 128]])
            pend_z2b = None  # (z2b, gi) transposed one group later

            def emit_z2bT(z2b, gi):
                z2bT_ps = psum.tile([128, 4, 128], bf16, tag="tp", bufs=2,
                                    name="z2bT_ps")
                for tau in range(4):
                    nc.tensor.transpose(z2bT_ps[:, tau, :],
                                        z2b[:, 128 * tau:128 * (tau + 1)],
                                        ident128[:, :])
                nc.scalar.activation(out=z2bT_ch[:, :, gi * 128:(gi + 1) * 128],
                                     in_=z2bT_ps, func=AF.Copy)

            for gi in range(gpc):
                pg2 = pool.tile([128, 4, 128], bf16, tag="pg2", bufs=2,
                                name="pg2")
                halves = []
                for half in range(2):
                    sb0 = gi * 8 + half * 4
                    # scores: 4 quadrant matmuls per sub-group. Split into
                    # two PSUM tiles by column half (h2) so concurrent MMs
                    # on different PE row-groups never drain to the same
                    # PSUM partitions of one bank (HW write collision).
                    s2L = psum.tile([128, 4, 64], f32, tag="att", bufs=3,
                                    name="s2L")
                    s2R = psum.tile([128, 4, 64], f32, tag="att", bufs=3,
                                    name="s2R")
                    for u in range(4):
                        sb = sb0 + u
                        for g2, h2 in ((0, 0), (1, 1), (1, 0), (0, 1)):
                            kX = kTb if g2 == h2 else kTw
                            off = h2 * 64 * QP + sb * 64
                            stat = apx(kX, off, [[QP, 64], [1, 64]])
                            mov = apx(qTb, off, [[QP, 64], [1, 64]])
                            s2 = s2L if h2 == 0 else s2R
                            nc.tensor.matmul(
                                s2[g2 * 64:(g2 + 1) * 64, u, :],
                                stat, mov, start=True, stop=True)
                    e_sb = pool.tile([128, 4, 128], bf16, tag="esb", bufs=2,
                                     name="e_sb")
                    eL = apx(e_sb, 0, [[512, 128], [128, 4], [1, 64]])
                    eR = apx(e_sb, 64, [[512, 128], [128, 4], [1, 64]])
                    nc.scalar.activation(out=eL, in_=s2L, func=AF.Exp)
                    nc.scalar.activation(out=eR, in_=s2R, func=AF.Exp)
                    a_sb = pool.tile([128, 4, 128], bf16, tag="asb", bufs=2,
                                     name="a_sb")
                    nc.gpsimd.tensor_tensor(out=a_sb, in0=e_sb, in1=mask_b,
                                            op=AL.mult)
                    # v sub-group packing via per-half transposes
                    vt_ps = psum.tile([128, 4, 64], bf16, tag="tp", bufs=2,
                                      name="vt_ps")
                    for u in range(4):
                        sb = sb0 + u
                        for g2 in range(2):
                            src = apx(vTb, g2 * 64 * QP + sb * 64,
                                      [[QP, 64], [1, 64]])
                            nc.tensor.transpose(
                                vt_ps[g2 * 64:(g2 + 1) * 64, u, :], src,
                                ident128[g2 * 64:(g2 + 1) * 64,
                                         g2 * 64:(g2 + 1) * 64])
                    vaug = vaugA if half == 0 else vaugB
                    nc.vector.tensor_copy(out=vaug[:, :, 0:64], in_=vt_ps)
                    halves.append((a_sb, vaug))

                for half in range(2):
                    a_sb, vaug = halves[half]
                    # AV (+denominator)
                    outS = psum.tile([128, 4, 66], f32, tag="att", bufs=3,
                                     name="outS")
                    for u in range(4):
                        nc.tensor.matmul(outS[:, u, 0:65], a_sb[:, u, :],
                                         vaug[:, u, 0:65], start=True,
                                         stop=True)
                    # dense-MM filler hides this half's normalize chain
                    if fill_i[0] < min(len(fillers), 8):
                        fillers[fill_i[0]]()
                        fill_i[0] += 1
                    # normalize
                    recip = pool.tile([128, 4], f32, tag="recip", bufs=2,
                                      name="recip")
                    nc.vector.reciprocal(out=recip, in_=outS[:, :, 64])
                    ogb = pool.tile([128, 4, 64], bf16, tag="ogb", bufs=2,
                                    name="ogb")
                    nc.vector.tensor_tensor(out=ogb, in0=outS[:, :, 0:64],
                                            in1=bcast_u(recip, 4, 64),
                                            op=AL.mult)
                    # out transposes split by head parity
                    # -> pg2[(h2,d), h1, (sb,j)]
                    p_ps = psum.tile([128, 4, 64], bf16, tag="tp", bufs=2,
                                     name="p_ps")
                    for u in range(4):
                        for h2 in range(2):
                            nc.tensor.transpose(
                                p_ps[h2 * 64:(h2 + 1) * 64, u, :],
                                ogb[h2 * 64:(h2 + 1) * 64, u, :],
                                ident128[h2 * 64:(h2 + 1) * 64,
                                         h2 * 64:(h2 + 1) * 64])
                    # pg2 col = h1*128 + sb*16 + j (per-h1 contiguous for
                    # the single-free-dim O-proj stationary)
                    pg2_ap = apx(pg2, half * 64,
                                 [[512, 128], [16, 4], [128, 4], [1, 16]])
                    nc.scalar.activation(out=pg2_ap, in_=p_ps, func=AF.Copy)

                # ---- O projection: K=128 over (h2, d), accumulate over h1 ----
                oproj_ps = psum.tile([128, E], f32, tag="big", bufs=3,
                                     name="oproj_ps")
                for h1 in range(4):
                    nc.tensor.matmul(oproj_ps, pg2[:, h1, :], rwo2_sb[:, h1, :],
                                     start=(h1 == 0),
                                     stop=(h1 == 3 and not has_bo))
                if has_bo:
                    nc.tensor.matmul(oproj_ps, ones1_sb, bo_sb,
                                     start=False, stop=True)

                # ---- residual 1 + LN2 (z2bT transposes deferred a group) ----
                x2_sb = pool.tile([128, E], f32, tag="x2", bufs=2 * gpc + 2,
                                  name="x2_sb")
                nc.vector.tensor_add(out=x2_sb, in0=x_tiles[gi], in1=oproj_ps)
                x2_tiles.append(x2_sb)
                z2b = layernorm_to_bf16(x2_sb, "ln2")
                if pend_z2b is not None:
                    emit_z2bT(*pend_z2b)
                pend_z2b = (z2b, gi)
                # next chunk's x load + LN1 chain, spread across groups
                if next_c is not None:
                    nxt.append(emit_ln1_group(next_c, gi))

            if len(fillers) > 8:
                fillers[8]()
                emit_z2bT(*pend_z2b)
                fillers[9]()
            else:
                emit_z2bT(*pend_z2b)
            return x2_tiles, z2bT_ch, nxt

        def make_ffn_pieces(c, z2bT_ch, x2_tiles):
            """FFN for chunk c as 6 closures: 4x (u1 ft block), u2, tail."""
            cell = {}

            def u1_block(ft0):
                def go():
                    if ft0 == 0:
                        cell["rT"] = pool.tile([128, 16, npcch], f8,
                                               tag="rt", bufs=1, name="rT_sb")
                    rT_sb = cell["rT"]
                    for ft in range(ft0, ft0 + 2):
                        u1_ps = psum.tile([128, npcch], f32, tag="big",
                                          bufs=3, name="u1_ps")
                        for t2 in range(2):
                            stat = apx(rw1_sb, t2 * 2 * F + ft * 128,
                                       [[4 * F, 128], [F, 2], [1, 128]])
                            mov = apx(z2bT_ch, t2 * 2 * npcch,
                                      [[4 * npcch, 128], [npcch, 2],
                                       [1, npcch]])
                            nc.tensor.matmul(u1_ps, stat, mov, perf_mode=DR,
                                             start=(t2 == 0), stop=(t2 == 1))
                        if has_c2f:
                            nc.vector.tensor_scalar(
                                out=rT_sb[:, ft, :], in0=u1_ps,
                                scalar1=c2f_sb[:, ft:ft + 1],
                                scalar2=0.0, op0=AL.add, op1=AL.max)
                        elif ft % 2 == 0:
                            nc.scalar.activation(out=rT_sb[:, ft, :],
                                                 in_=u1_ps, func=AF.Relu)
                        else:
                            nc.vector.tensor_scalar_max(out=rT_sb[:, ft, :],
                                                        in0=u1_ps, scalar1=0.0)
                return go

            def u2_all():
                rT_sb = cell["rT"]
                u2b_sb = pool.tile([128, 4, npcch], bf16, tag="u2b", bufs=2,
                                   name="u2b_sb")
                cell["u2b"] = u2b_sb
                for et in range(4):
                    u2_ps = psum.tile([128, npcch], f32, tag="big", bufs=3,
                                      name="u2_ps")
                    for t2 in range(8):
                        stat = apx(w2t_sb, t2 * 2 * E + et * 128,
                                   [[16 * E, 128], [E, 2], [1, 128]])
                        mov = apx(rT_sb, t2 * 2 * npcch,
                                  [[16 * npcch, 128], [npcch, 2], [1, npcch]])
                        nc.tensor.matmul(u2_ps, stat, mov, perf_mode=DR,
                                         start=(t2 == 0), stop=(t2 == 7))
                    if has_b2:
                        nc.vector.tensor_scalar_add(
                            out=u2b_sb[:, et, :], in0=u2_ps,
                            scalar1=b2_sb[:, et:et + 1])
                    elif et % 2 == 0:
                        nc.scalar.activation(out=u2b_sb[:, et, :], in_=u2_ps,
                                             func=AF.Copy)
                    else:
                        nc.vector.tensor_copy(out=u2b_sb[:, et, :], in_=u2_ps)

            def tail():
                u2b_sb = cell["u2b"]
                for gi in range(gpc):
                    g = c * gpc + gi
                    u2n_ps = psum.tile([128, 4, 128], bf16, tag="tp", bufs=2,
                                       name="u2n_ps")
                    for et in range(4):
                        nc.tensor.transpose(
                            u2n_ps[:, et, :],
                            u2b_sb[:, et, 128 * gi:128 * (gi + 1)],
                            ident128[:, :])
                    u2nat = pool.tile([128, 4, 128], bf16, tag="u2nat",
                                      bufs=2, name="u2nat")
                    if gi % 2 == 0:
                        nc.vector.tensor_copy(out=u2nat, in_=u2n_ps)
                    else:
                        nc.scalar.activation(out=u2nat, in_=u2n_ps,
                                             func=AF.Copy)
                    out_sb = pool.tile([128, E], f32, tag="osb", bufs=3,
                                       name="out_sb")
                    nc.vector.tensor_add(out=out_sb, in0=x2_tiles[gi],
                                         in1=u2nat)
                    nc.sync.dma_start(out=out_d[g * 128:(g + 1) * 128, :],
                                      in_=out_sb)

            return [u1_block(ft0) for ft0 in range(0, 16, 2)] + [u2_all, tail]

        # ---- software-pipelined main loop (chunk-skewed, interleaved FFN) --
        x_next, zb_next = emit_ln1_loads(0)
        zbT_next = emit_zbT(zb_next)
        fillers = ()
        for c in range(n_chunks):
            x_cur, zbT_cur = x_next, zbT_next
            nc_arg = c + 1 if c + 1 < n_chunks else None
            x2_tiles, z2bT_ch, nxt = emit_chunk(c, x_cur, zbT_cur, fillers,
                                                next_c=nc_arg)
            if nc_arg is not None:
                x_next = [t[0] for t in nxt]
                zbT_next = emit_zbT([t[1] for t in nxt])
            fillers = make_ffn_pieces(c, z2bT_ch, x2_tiles)
        for piece in fillers:
            piece()

    if fix_waits:
        _fix_sync_waits(nc)


def _fix_sync_waits(nc):
    """walrus limits inline sync waits to 1 per instruction. Tile can
    emit more. Split the excess into standalone InstEventSemaphore
    wait-carriers inserted immediately before the overweight instruction
    on the same engine - semantically identical (the waits still execute
    right before the instruction, in order)."""
    import concourse.mybir as mybir
    n = 0
    for f in nc.m.functions:
        for blk in f.blocks:
            insts = blk.instructions
            out = []
            dirty = False
            for inst in insts:
                si = inst.sync_info
                waits = list(si.on_wait) if (si and si.on_wait) else []
                limit = 1
                if len(waits) > limit:
                    ups = list(si.on_update) if (si and si.on_update) else []
                    up_ids = {u.id for u in ups}
                    # keep own-queue credit waits inline (DMA flow control)
                    waits.sort(key=lambda w: 0 if w.id in up_ids else 1)
                    keep, move = waits[:limit], waits[limit:]
                    for w in move:
                        n += 1
                        car = mybir.InstEventSemaphore(
                            name="WSPLIT-%d" % n, ins=[], outs=[])
                        car.engine = inst.engine
                        car.sync_info = mybir.SyncInfo(on_wait=[w],
                                                       on_update=[])
                        out.append(car)
                    inst.sync_info = mybir.SyncInfo(on_wait=keep,
                                                   on_update=ups)
                    dirty = True
                out.append(inst)
            if dirty:
                blk.instructions = out
    return n


def _prep_weights(inputs):
    """Host-side weight folding. Returns dict of np arrays + flags."""
    f32 = np.float32
    g1 = np.asarray(inputs["g1"], f32)
    beta1 = np.asarray(inputs["beta1"], f32)
    g2 = np.asarray(inputs["g2"], f32)
    beta2 = np.asarray(inputs["beta2"], f32)
    Wq = np.asarray(inputs["Wq"], f32)
    Wk = np.asarray(inputs["Wk"], f32)
    Wv = np.asarray(inputs["Wv"], f32)
    Wo = np.asarray(inputs["Wo"], f32)
    W1 = np.asarray(inputs["W1"], f32)
    W2 = np.asarray(inputs["W2"], f32)
    scale = np.float32(1.0 / np.sqrt(D))

    rwq = (Wq.T * g1[:, None] * scale).astype(BF)
    rwk = (Wk.T * g1[:, None]).astype(BF)
    rwv = (Wv.T * g1[:, None]).astype(BF)
    rw1 = (W1.T * g2[:, None]).astype(F8)
    w2t = W2.T.astype(F8)

    # column-swapped K weights: within each 128-col block, swap 64-halves
    sw = np.arange(E)
    sw = (sw // 128) * 128 + (sw % 128 + 64) % 128
    rwkw = np.ascontiguousarray(rwk[:, sw])

    # rwo2[(h2*64+d), h1, fo] = Wo[fo, (2*h1+h2)*64+d]  (stored [E, E] with
    # row index (h2*64+d)*4... laid out as [(p t) e] with p=(h2*64+d), t=h1)
    WoT = Wo.T.astype(BF)  # [E(h*64+d), E(fo)]
    rwo2 = np.zeros((E, E), BF)
    for h2 in range(2):
        for h1 in range(4):
            h = 2 * h1 + h2
            # destination rows (h2*64+d) at t=h1: flat row = (h2*64+d)*4 + h1
            rwo2[(np.arange(64) + h2 * 64) * 4 + h1, :] = WoT[h * 64:(h + 1) * 64, :]

    c2q = ((Wq @ beta1 + np.asarray(inputs["bq"], f32)) * scale).astype(f32)
    c2k = (Wk @ beta1 + np.asarray(inputs["bk"], f32)).astype(f32)
    c2kw = np.ascontiguousarray(c2k[sw])
    c2v = (Wv @ beta1 + np.asarray(inputs["bv"], f32)).astype(f32)
    bo = np.asarray(inputs["bo"], f32)
    c2f = (W1 @ beta2 + np.asarray(inputs["b1"], f32)).astype(f32)
    b2 = np.asarray(inputs["b2"], f32)

    mask = np.zeros((128, 128), f32)
    for i in range(16):
        for gg in range(8):
            for hh in range(8):
                mask[gg * 16 + i, hh * 16 + i] = 1.0

    return dict(
        rwq=rwq, rwk=rwk, rwkw=rwkw, rwv=rwv, rwo2=rwo2, rw1=rw1, w2t=w2t,
        mask=mask.astype(BF),
        c2q=c2q, c2k=c2k, c2kw=c2kw, c2v=c2v, bo=bo.astype(BF), c2f=c2f, b2=b2,
        has_qkv_bias=bool(np.any(c2q) or np.any(c2k) or np.any(c2v)),
        has_bo=bool(np.any(bo)), has_c2f=bool(np.any(c2f)),
        has_b2=bool(np.any(b2)),
    )


def kernel(**inputs):
    from concourse.bass_utils import run_bass_kernel_spmd

    x = np.asarray(inputs["x"], np.float32)
    n = x.shape[0]
    npc = n // N_CORES
    w = _prep_weights(inputs)

    nc = build_nc(npc, has_qkv_bias=w["has_qkv_bias"], has_bo=w["has_bo"],
                  has_c2f=w["has_c2f"], has_b2=w["has_b2"])

    shared = dict(rwq=w["rwq"], rwk=w["rwk"], rwkw=w["rwkw"], rwv=w["rwv"],
                  rwo2=w["rwo2"], rw1=w["rw1"], w2t=w["w2t"], mask=w["mask"],
                  c2q=w["c2q"], c2k=w["c2k"], c2kw=w["c2kw"], c2v=w["c2v"],
                  bo=w["bo"], c2f=w["c2f"], b2=w["b2"])
    in_maps = []
    for core in range(N_CORES):
        m = dict(shared)
        m["x"] = np.ascontiguousarray(x[core * npc:(core + 1) * npc])
        in_maps.append(m)

    res = run_bass_kernel_spmd(nc, in_maps, list(range(N_CORES)))
    out = np.concatenate([np.asarray(res.results[c]["out"])
                          for c in range(N_CORES)], axis=0)
    return out.astype(np.float32)
